# revision 1
# baseline (speedup 1.0000x reference)
"""DGCNN encoder (4x EdgeConv + global max) as a Bass/Tile kernel on 8 TRN2 cores.

Sharding: data-parallel over batch B=4 with a 2-way query split per cloud.
Core c handles cloud c//2. The host rotates each cloud's points by 1024 for
odd cores, so every core runs the SAME program: layers 1-3 are computed for
all 2048 points (needed for the next layer's kNN), layer 4 and the partial
channel max only for the first 1024 points (= this core's half). The final
(B,1,512) is a host-side max of the two per-core partial maxima per cloud.

Per-layer device algorithm (EdgeConv restructured, channel-transposed):
  dist score q[n,j] = 2<f_n,f_j> - |f_j|^2  (selection-equivalent to the
  reference's top_k ordering; one PE matmul with an augmented contraction row)
  -> exact top-20 per row via 3 rounds of DVE max8/match_replace/max_index
  -> zT = (s*Wn) @ f^T, yT = (s*Wc) @ f^T  (BN scale s folded on host)
  -> MT[o,n] = max_k zT[o,idx[n,k]] via gpsimd indirect_copy column gathers
     (1024 idxs per call, k-major) + DVE max accumulation
  -> outT = max(pre, 0.2*pre), pre = MT - zT + yT  (LeakyReLU(0.2); s>0)
outT is directly the next layer's featT (no transposes anywhere).
"""
import numpy as np

import concourse.bass as bass
import concourse.tile as tile
from concourse import bacc, mybir
from concourse.bass_utils import run_bass_kernel_spmd

F32 = mybir.dt.float32
F16 = mybir.dt.float16
U16 = mybir.dt.uint16

N = 2048          # points per cloud
NQ4 = 1024        # layer-4 query half
K = 20            # neighbors
P = 128           # partitions
NT = N // P       # 16 row tiles
NT4 = NQ4 // P    # 8 row tiles for layer 4
GCH = 1024        # indirect_copy index-chunk size (HW limit)
NEG = -1.0e30
LAYERS = [
    # (Cin, Cout, n_qtiles)
    (3, 64, NT),
    (64, 64, NT),
    (64, 128, NT),
    (128, 256, NT4),
]


def _topk_rounds(nc, pool, q_ap, nbr16, t):
    """Exact top-20 (as top-24, first 20 valid) of each row of q_ap (P, N).

    q_ap lives in PSUM; match_replace writes SBUF copies. Indices (uint16,
    descending by value) land in nbr16[:, 0:24, t] (k-contiguous layout).
    """
    v8 = pool.tile([P, 24], F32, name="v8", tag="v8")
    qm = pool.tile([P, N], F32, name="qm", tag="qmod")
    cur = q_ap
    for r in range(3):
        vr = v8[:, r * 8 : (r + 1) * 8]
        nc.vector.max(out=vr, in_=cur)
        nc.vector.max_index(
            out=nbr16[:, r * 8 : (r + 1) * 8, t], in_max=vr, in_values=q_ap
        )
        if r < 2:
            nc.vector.match_replace(
                out=qm[:], in_to_replace=vr, in_values=cur, imm_value=NEG
            )
            cur = qm[:]


def build_program():
    nc = bacc.Bacc("TRN2", target_bir_lowering=False, debug=False, num_devices=8)

    featT1 = nc.declare_dram_parameter("featT1", [4, N], F32, isOutput=False)
    augK1 = nc.declare_dram_parameter("augK1", [4, N], F32, isOutput=False)
    wnT = {}
    wcT = {}
    for li, (C, O, _) in enumerate(LAYERS, start=1):
        wnT[li] = nc.declare_dram_parameter(f"wnT{li}", [C, O], F32, isOutput=False)
        wcT[li] = nc.declare_dram_parameter(f"wcT{li}", [C, O], F32, isOutput=False)
    pmax_out = nc.declare_dram_parameter("pmax", [P, 5], F32, isOutput=True)

    with tile.TileContext(nc) as tc:
        _build_tc(tc, featT1, augK1, wnT, wcT, pmax_out)
    return nc


def _build_tc(tc, featT1_d, augK1_d, wnT_d, wcT_d, pmax_d):
    nc = tc.nc
    import contextlib

    with contextlib.ExitStack() as ctx:
        const = ctx.enter_context(tc.tile_pool(name="const", bufs=1))
        feats = ctx.enter_context(tc.tile_pool(name="feats", bufs=3))
        auks = ctx.enter_context(tc.tile_pool(name="auks", bufs=3))
        sc2k = ctx.enter_context(tc.tile_pool(name="sc2k", bufs=1))
        zpool = ctx.enter_context(tc.tile_pool(name="zpool", bufs=2))
        gzp = ctx.enter_context(tc.tile_pool(name="gzp", bufs=3))
        small = ctx.enter_context(tc.tile_pool(name="small", bufs=2))
        qpsum = ctx.enter_context(tc.tile_pool(name="qpsum", bufs=1, space="PSUM"))
        zypsum = ctx.enter_context(tc.tile_pool(name="zypsum", bufs=2, space="PSUM"))
        xxpsum = ctx.enter_context(tc.tile_pool(name="xxpsum", bufs=1, space="PSUM"))

        ones_col = const.tile([P, 1], F32, name="ones_col")
        nc.vector.memset(ones_col[:], 1.0)
        pm = const.tile([P, 5], F32, name="pm")
        nc.vector.memset(pm[:], NEG)

        featT1 = feats.tile([4, N], F32, name="featT1", tag="ft_small")
        nc.sync.dma_start(featT1[:], featT1_d[:])
        augK1 = auks.tile([4, N], F32, name="augK1", tag="ak_small")
        nc.sync.dma_start(augK1[:], augK1_d[:])

        # weights: (C, O); O>128 split into output halves, C>64 split into
        # contraction parts (separate tiles so each starts at base partition 0)
        wn_sb = {}
        wc_sb = {}
        for li, (C, O, _) in enumerate(LAYERS, start=1):
            nh = (O + P - 1) // P
            csplits = [(0, C)] if C <= 64 else [(0, 64), (64, C)]
            wn_sb[li] = []
            wc_sb[li] = []
            for h in range(nh):
                o0, o1 = h * P, min((h + 1) * P, O)
                wn_sb[li].append([])
                wc_sb[li].append([])
                for ci, (c0, c1) in enumerate(csplits):
                    wt = const.tile([c1 - c0, o1 - o0], F32, name=f"wn{li}_{h}_{ci}")
                    nc.sync.dma_start(wt[:], wnT_d[li][c0:c1, o0:o1])
                    wn_sb[li][h].append(wt)
                    wt2 = const.tile([c1 - c0, o1 - o0], F32, name=f"wc{li}_{h}_{ci}")
                    nc.sync.dma_start(wt2[:], wcT_d[li][c0:c1, o0:o1])
                    wc_sb[li][h].append(wt2)

        # feat_state: zy = list of lhsT contraction parts (each base partition 0)
        #             dist = list of (lhsT_ap, rhs_ap) contraction parts
        feat_state = {
            "zy": [featT1[0:3, :]],
            "dist": [(featT1[0:4, :], augK1[0:4, :])],
        }

        for li, (C, O, NQT) in enumerate(LAYERS, start=1):
            last = li == 4
            NQ = NQT * P
            nh = (O + P - 1) // P  # output-channel halves (2 for L4)

            # ------- zT / yT matmuls: (O, N) channel-major -------
            zT = []
            yT = []
            for h in range(nh):
                o0, o1 = h * P, min((h + 1) * P, O)
                oc = o1 - o0
                zt = zpool.tile([P, N], F16, name=f"zT{li}_{h}", tag=f"z{h}")
                yt = zpool.tile([P, NQ], F32, name=f"yT{li}_{h}", tag=f"y{h}")
                if oc < P:
                    nc.vector.memset(zt[:], 0.0)
                zT.append(zt)
                yT.append(yt)
                for t in range(NT):
                    zp = zypsum.tile([oc, P], F32, name=f"zp{li}_{h}_{t}", tag="zy")
                    for pi, lhs in enumerate(feat_state["zy"]):
                        nc.tensor.matmul(
                            zp[:],
                            wn_sb[li][h][pi][:],
                            lhs[:, t * P : (t + 1) * P],
                            start=(pi == 0),
                            stop=(pi == len(feat_state["zy"]) - 1),
                        )
                    nc.scalar.copy(zt[0:oc, t * P : (t + 1) * P], zp[:])
                    if t < NQT:
                        yp = zypsum.tile([oc, P], F32, name=f"yp{li}_{h}_{t}",
                                         tag="zy")
                        for pi, lhs in enumerate(feat_state["zy"]):
                            nc.tensor.matmul(
                                yp[:],
                                wc_sb[li][h][pi][:],
                                lhs[:, t * P : (t + 1) * P],
                                start=(pi == 0),
                                stop=(pi == len(feat_state["zy"]) - 1),
                            )
                        # c = y - z on own queries (z via its fp16 SBUF copy;
                        # DVE may read only one PSUM operand per instruction)
                        nc.vector.tensor_sub(
                            yt[0:oc, t * P : (t + 1) * P],
                            yp[:],
                            zt[0:oc, t * P : (t + 1) * P],
                        )

            # ------- dist + topk -------
            nbr16 = small.tile([P, 24, NQT], U16, name=f"nbr{li}", tag="nbr")
            for t in range(NQT):
                q = qpsum.tile([P, N], F32, name=f"q{li}_{t}", tag="q")
                for ch in range(4):
                    cs = bass.ts(ch, 512)
                    nparts = len(feat_state["dist"])
                    for pi, (lhsT, rhs) in enumerate(feat_state["dist"]):
                        nc.tensor.matmul(
                            q[:, cs],
                            lhsT[:, t * P : (t + 1) * P],
                            rhs[:, cs],
                            start=(pi == 0),
                            stop=(pi == nparts - 1),
                        )
                # free the single PSUM q buffer fast: copy scores to SBUF so
                # the next tile's dist matmul (PE) overlaps this topk (DVE)
                qsb = small.tile([P, N], F32, name=f"qsb{li}_{t}", tag="qsb")
                nc.scalar.copy(qsb[:], q[:])
                _topk_rounds(nc, small, qsb[:], nbr16, t)

            # ------- gather index shuffle to flat k-major (i = k*NQ + n) -------
            # entry (p,t,k): flat i = k*NQ + t*128 + p -> row p%16 (+16*rep),
            # col k*(NQT*8) + t*8 + p//16
            gidx = small.tile([P, K * NQ // 16], U16, name=f"gidx{li}", tag="gidx")
            for g in range(8):
                src = nbr16[g * 16 : (g + 1) * 16, 0:K, :].rearrange("p k t -> p (k t)")
                dst = gidx[0:16, :].rearrange("p (kt g2) -> p kt g2", g2=8)[:, :, g]
                nc.sync.dma_start(dst, src)
            for rep in range(1, 8):
                nc.sync.dma_start(gidx[rep * 16 : (rep + 1) * 16, :], gidx[0:16, :])

            # ------- gather + max over k (per 1024-point chunk, per k) -------
            MT = []
            for h in range(nh):
                for half in range(NQ // GCH):
                    m = zpool.tile(
                        [P, GCH], F32, name=f"MT{li}_{h}_{half}", tag=f"M{h}_{half}"
                    )
                    for k in range(K):
                        i0 = k * NQ + half * GCH
                        gz = gzp.tile([P, GCH], F16, name=f"gz{li}", tag="gz")
                        nc.gpsimd.indirect_copy(
                            out=gz[:],
                            data=zT[h][:],
                            idxs=gidx[:, i0 // 16 : (i0 + GCH) // 16],
                            i_know_ap_gather_is_preferred=True,
                        )
                        if k == 0:
                            nc.vector.tensor_copy(out=m[:], in_=gz[:])
                        else:
                            nc.vector.tensor_max(m[:], m[:], gz[:])
                    MT.append((h, half, m))

            # ------- combine: out = lrelu(M - z + y) -------
            if not last:
                C2 = O
                if C2 <= 64:
                    ft = feats.tile([C2 + 1, N], F32, name=f"featT{li+1}a",
                                    tag="ft_small")
                    ft_parts = [(ft, 0, C2)]
                else:
                    fta = feats.tile([64, N], F32, name=f"featT{li+1}a",
                                     tag="ft_small")
                    ftb = feats.tile([C2 - 64 + 1, N], F32, name=f"featT{li+1}b",
                                     tag="ft_b", bufs=1)
                    ft = fta
                    ft_parts = [(fta, 0, 64), (ftb, 64, C2)]

            for h, half, m in MT:
                o0, o1 = h * P, min((h + 1) * P, O)
                oc = o1 - o0
                cslice = slice(half * GCH, (half + 1) * GCH)
                nc.vector.tensor_add(m[0:oc, :], m[0:oc, :], yT[h][0:oc, cslice])
                sc = gzp.tile([P, GCH], F32, name=f"sc{li}", tag="sc")
                nc.scalar.mul(sc[0:oc, :], m[0:oc, :], 0.2)
                nc.vector.tensor_max(m[0:oc, :], m[0:oc, :], sc[0:oc, :])
                # partial channel max over own points (first 1024 columns)
                if half == 0:
                    col = {1: 0, 2: 1, 3: 2}.get(li, 3 + h)
                    nc.vector.tensor_reduce(
                        out=pm[0:oc, col : col + 1],
                        in_=m[0:oc, :],
                        axis=mybir.AxisListType.X,
                        op=mybir.AluOpType.max,
                    )
                if not last:
                    for buf, r0, r1 in ft_parts:
                        rr0 = max(r0, o0)
                        rr1 = min(r1, o1)
                        if rr0 >= rr1:
                            continue
                        if rr0 - r0 == rr0 - o0:
                            nc.vector.tensor_copy(
                                out=buf[rr0 - r0 : rr1 - r0, cslice],
                                in_=m[rr0 - o0 : rr1 - o0, :],
                            )
                        else:
                            # partition-base shift (e.g. rows 64:128 -> 0:64)
                            nc.sync.dma_start(
                                buf[rr0 - r0 : rr1 - r0, cslice],
                                m[rr0 - o0 : rr1 - o0, :],
                            )

            if last:
                break

            # ------- next-layer augK + xxrow -------
            C2 = O
            sq = sc2k.tile([C2, N], F32, name=f"sq{li}", tag="sc2k")
            for buf, r0, r1 in ft_parts:
                nc.scalar.square(sq[r0:r1, :], buf[0 : r1 - r0, :])
            if C2 <= 64:
                ak = auks.tile([C2 + 1, N], F32, name=f"augK{li+1}a", tag="ak_small")
            else:
                aka = auks.tile([64, N], F32, name=f"augK{li+1}a", tag="ak_small")
                akb = auks.tile([C2 - 64 + 1, N], F32, name=f"augK{li+1}b",
                                tag="ak_b", bufs=1)
            for ch in range(4):
                cs = bass.ts(ch, 512)
                xp = xxpsum.tile([1, 512], F32, name=f"xx{li}_{ch}", tag="xx")
                nc.tensor.matmul(xp[:], ones_col[0:C2, :], sq[:, cs])
                if C2 <= 64:
                    nc.scalar.copy(ak[C2 : C2 + 1, cs], xp[:])
                else:
                    nc.scalar.copy(akb[C2 - 64 : C2 - 64 + 1, cs], xp[:])
            if C2 <= 64:
                nc.scalar.mul(ak[0:C2, :], ft[0:C2, :], 2.0)
                nc.vector.memset(ft[C2 : C2 + 1, :], -1.0)
                feat_state = {
                    "zy": [ft[0:C2, :]],
                    "dist": [(ft[0 : C2 + 1, :], ak[0 : C2 + 1, :])],
                }
            else:
                nc.scalar.mul(aka[:], fta[0:64, :], 2.0)
                nc.scalar.mul(akb[0 : C2 - 64, :], ftb[0 : C2 - 64, :], 2.0)
                nc.vector.memset(ftb[C2 - 64 : C2 - 64 + 1, :], -1.0)
                feat_state = {
                    "zy": [fta[0:64, :], ftb[0 : C2 - 64, :]],
                    "dist": [(fta[0:64, :], aka[:]), (ftb[:], akb[:])],
                }

        nc.sync.dma_start(pmax_d[:], pm[:])


def _crow(pi, parts):
    r0 = sum(p.shape[0] for p in parts[:pi])
    return slice(r0, r0 + parts[pi].shape[0])


_NC_CACHE = None
TRACE = False          # set True (e.g. from test.py) to profile the HW run
RUN_KWARGS = {}        # extra kwargs for run_bass_kernel_spmd when tracing
LAST_RESULTS = None    # BassKernelResults of the most recent run


def _get_program():
    global _NC_CACHE
    if _NC_CACHE is None:
        nc = build_program()
        nc.finalize()   # bacc passes: library loads, act tables, ISA codegen
        _NC_CACHE = nc
    return _NC_CACHE


def kernel(**inputs) -> np.ndarray:
    x = np.asarray(inputs["x"], dtype=np.float32)       # (4, 2048, 3)
    B = x.shape[0]
    ws = {i: np.asarray(inputs[f"w{i}"], np.float32) for i in (1, 2, 3, 4)}
    EPS = 1e-5
    in_maps = []
    wmats = {}
    for li, (C, O, _) in enumerate(LAYERS, start=1):
        g = np.asarray(inputs[f"g{li}"], np.float64)
        b = np.asarray(inputs[f"b{li}"], np.float64)
        m = np.asarray(inputs[f"m{li}"], np.float64)
        v = np.asarray(inputs[f"v{li}"], np.float64)
        s = (g / np.sqrt(v + EPS)).astype(np.float32)
        t = (b - m * s).astype(np.float32)
        assert np.all(s > 0) and np.allclose(t, 0.0), "kernel assumes BN shift==0, scale>0"
        w = ws[li] * s[:, None]                           # fold BN scale
        wmats[li] = (
            np.ascontiguousarray(w[:, :C].T),             # wnT (C, O)
            np.ascontiguousarray(w[:, C:].T),             # wcT (C, O)
        )

    for core in range(8):
        b = core // 2
        roll = (core % 2) * NQ4
        xp = np.concatenate([x[b, roll:], x[b, :roll]], axis=0)  # (2048, 3)
        xx = np.sum(xp.astype(np.float32) ** 2, axis=1)
        featT1 = np.concatenate(
            [xp.T, np.full((1, N), -1.0, np.float32)], axis=0
        ).astype(np.float32)
        augK1 = np.concatenate([2.0 * xp.T, xx[None, :]], axis=0).astype(np.float32)
        im = {"featT1": featT1, "augK1": augK1}
        for li in (1, 2, 3, 4):
            im[f"wnT{li}"] = wmats[li][0]
            im[f"wcT{li}"] = wmats[li][1]
        in_maps.append(im)

    nc = _get_program()
    res = run_bass_kernel_spmd(
        nc, in_maps, core_ids=list(range(8)), trace=TRACE, **RUN_KWARGS
    )
    global LAST_RESULTS
    LAST_RESULTS = res

    # reassemble: pm cols = [x1(64), x2(64), x3(128), x4a(128), x4b(128)]
    out = np.empty((B, 1, 512), np.float32)
    for b in range(B):
        vs = []
        for core in (2 * b, 2 * b + 1):
            pmv = res.results[core]["pmax"]
            vs.append(
                np.concatenate(
                    [pmv[0:64, 0], pmv[0:64, 1], pmv[0:128, 2], pmv[0:128, 3],
                     pmv[0:128, 4]]
                )
            )
        out[b, 0] = np.maximum(vs[0], vs[1])
    return out



# revision 32
# speedup vs baseline: 3.3896x; 3.3896x over previous
"""DGCNN encoder (4x EdgeConv + global max) as a Bass/Tile kernel on 8 TRN2 cores.

Sharding: data-parallel over batch B=4 with a 2-way query split per cloud.
Core c handles cloud c//2. The host rotates each cloud's points by 1024 for
odd cores, so every core runs the SAME program: layers 1-3 are computed for
all 2048 points (needed for the next layer's kNN), layer 4 and the partial
channel max only for the first 1024 points (= this core's half). The final
(B,1,512) is a host-side max of the two per-core partial maxima per cloud.

Per-layer device algorithm (v2 — dma_gather + packed-index top-k):
  dist score q[n,j] = 2<f_n,f_j> - |f_j|^2  (selection-equivalent to the
  reference's top_k ordering; PE matmul with an augmented contraction row)
  -> pack point index into the low 11 mantissa bits of each fp32 score with
     ONE fused DVE op: qpk = (q & 0xFFFFF800) | iota  (scalar_tensor_tensor)
  -> exact top-20 via 3 rounds of DVE max8 (+ match_replace between rounds);
     indices come for free from the low bits of the max8 values (no
     find_index8 passes). 5 DVE passes/tile instead of 8.
  -> zRows (point-major (N, O) f16, padded rows of 512B stripes) is computed
     directly by PE matmuls (featT_tile^T @ WnT) — the gather source.
  -> neighbor gather+transpose runs on the 16 SDMA engines via
     gpsimd.dma_gather(transpose=True, SBUF source): ~1.8 ns/idx vs ~28 ns/idx
     for the old Q7 indirect_copy path. Output is channel-major (O, n_idx).
  -> M[o,n] = max_k z[o,idx[n,k]] via 20 strided DVE f16 maxes.
  -> y - z is ONE matmul with host-folded (Wc - Wn) weights; out =
     lrelu(M + (y-z)) via one DVE add + one fused mul/max (lrelu) pass.
"""
import numpy as np

import concourse.bass as bass
import concourse.tile as tile
from concourse import bacc, mybir
from concourse.bass_utils import run_bass_kernel_spmd

F32 = mybir.dt.float32
F16 = mybir.dt.float16
I32 = mybir.dt.int32
I16 = mybir.dt.int16

N = 2048          # points per cloud
K = 20            # neighbors
P = 128           # partitions
NEG = -1.0e30
AX = mybir.AxisListType.X
LAYERS = [
    # (Cin, Cout, n_qtiles)
    (3, 64, 16),
    (64, 64, 16),
    (64, 128, 16),
    (128, 256, 8),
]


def build_program():
    nc = bacc.Bacc("TRN2", target_bir_lowering=False, debug=False, num_devices=8)

    featT1 = nc.declare_dram_parameter("featT1", [5, N], F32, isOutput=False)
    augK1 = nc.declare_dram_parameter("augK1", [5, N], F32, isOutput=False)
    wn = {}
    wd = {}
    for li, (C, O, _) in enumerate(LAYERS, start=1):
        # wn: (C+2, O) with zero last rows (pair with featT's aug rows so the
        # zRows matmul can share the dist stationary parts)
        wn[li] = nc.declare_dram_parameter(f"wn{li}", [C + 2, O], F32, isOutput=False)
        # wd = (Wc - Wn)^T: y - z in one matmul
        wd[li] = nc.declare_dram_parameter(f"wd{li}", [C, O], F32, isOutput=False)
    ident = nc.declare_dram_parameter("ident", [P, P], F16, isOutput=False)
    pmax_out = nc.declare_dram_parameter("pmax", [P, 5], F32, isOutput=True)

    with tile.TileContext(nc) as tc:
        _build_tc(tc, featT1, augK1, wn, wd, ident, pmax_out)
    return nc


def _build_tc(tc, featT1_d, augK1_d, wn_d, wd_d, ident_d, pmax_d):
    nc = tc.nc
    import contextlib

    AND = mybir.AluOpType.bitwise_and
    OR = mybir.AluOpType.bitwise_or
    MUL = mybir.AluOpType.mult
    MAX = mybir.AluOpType.max

    with contextlib.ExitStack() as ctx:
        const = ctx.enter_context(tc.tile_pool(name="const", bufs=1))
        feats = ctx.enter_context(tc.tile_pool(name="feats", bufs=2))
        auks = ctx.enter_context(tc.tile_pool(name="auks", bufs=1))
        qpool = ctx.enter_context(tc.tile_pool(name="qpool", bufs=2))
        small = ctx.enter_context(tc.tile_pool(name="small", bufs=2))
        zrpool = ctx.enter_context(tc.tile_pool(name="zrpool", bufs=2))
        gzp = ctx.enter_context(tc.tile_pool(name="gzp", bufs=12))
        mpool = ctx.enter_context(tc.tile_pool(name="mpool", bufs=1))
        ypool = ctx.enter_context(tc.tile_pool(name="ypool", bufs=1))
        prep = ctx.enter_context(tc.tile_pool(name="prep", bufs=1))
        dramp = ctx.enter_context(tc.tile_pool(name="dramp", bufs=2, space="DRAM"))
        qpsum = ctx.enter_context(tc.tile_pool(name="qpsum", bufs=1, space="PSUM"))
        zypsum = ctx.enter_context(tc.tile_pool(name="zypsum", bufs=2, space="PSUM"))
        xxpsum = ctx.enter_context(tc.tile_pool(name="xxpsum", bufs=1, space="PSUM"))

        ones_col = const.tile([P, 1], F32, name="ones_col")
        nc.vector.memset(ones_col[:], 1.0)
        pm = const.tile([P, 5], F32, name="pm")
        nc.vector.memset(pm[:], NEG)
        mask_hi = const.tile([P, 1], I32, name="mask_hi")
        nc.vector.memset(mask_hi[:], -2048)        # 0xFFFFF800
        mask_lo = const.tile([P, 1], I32, name="mask_lo")
        nc.vector.memset(mask_lo[:], 2047)         # 0x000007FF
        zeros24 = const.tile([P, 24], I32, name="zeros24")
        nc.vector.memset(zeros24[:], 0)
        iota32 = const.tile([P, N], I32, name="iota32")
        nc.gpsimd.iota(iota32[:], pattern=[[1, N]], base=0, channel_multiplier=0)
        # dma_gather idx tensor: rows 0-15 data, 16-31 replica (Q7 pair for
        # queue 0); rows 32-127 must hold valid values for the simulator.
        idxL = const.tile([P, 2560], I16, name="idxL")
        nc.gpsimd.memset(idxL[:], 0)

        featT1 = const.tile([5, N], F32, name="featT1")
        nc.sync.dma_start(featT1[:], featT1_d[:])
        augK1 = const.tile([5, N], F32, name="augK1")
        nc.sync.dma_start(augK1[:], augK1_d[:])
        ident = const.tile([P, P], F16, name="ident")
        nc.sync.dma_start(ident[:], ident_d[:])

        # weights: contraction parts start at base partition 0 each
        wn_sb = {}
        wd_sb = {}
        for li, (C, O, _) in enumerate(LAYERS, start=1):
            if C <= 64:
                wsplit = [(0, C + 2)]
                dsplit = [(0, C)]
            else:
                wsplit = [(0, 64), (64, C + 2)]
                dsplit = [(0, 64), (64, C)]
            wn_sb[li] = []
            wd_sb[li] = []
            for ci, (c0, c1) in enumerate(wsplit):
                t = const.tile([c1 - c0, O], F32, name=f"wn{li}_{ci}")
                nc.sync.dma_start(t[:], wn_d[li][c0:c1, :])
                wn_sb[li].append(t)
            for ci, (c0, c1) in enumerate(dsplit):
                t = const.tile([c1 - c0, O], F32, name=f"wd{li}_{ci}")
                nc.sync.dma_start(t[:], wd_d[li][c0:c1, :])
                wd_sb[li].append(t)

        # dist: list of (lhsT_ap, rhs_ap) contraction parts. featT carries aug
        # rows (-1, xx[n]); augK carries (xx[j], -(1+eps)) so the matmul
        # yields q[n,j] = 2<f_n,f_j> - xx[j] - (1+eps)*xx[n] = -d^2 - eps*xx[n]
        # (row-constant shift: selection-equivalent, and top scores sit near 0
        # so clearing 11 mantissa bits for index packing costs ~2^-12 of the
        # per-candidate distance, not of the raw score magnitude).
        # feat: lhsT contraction parts without aug rows (for y-z)
        feat_state = {
            "dist": [(featT1[0:5, :], augK1[0:5, :])],
            "feat": [featT1[0:3, :]],
        }

        for li, (C, O, NQT) in enumerate(LAYERS, start=1):
            last = li == 4
            NQ = NQT * P
            nh = 2 if last else 1
            QT = NQT // 4            # query tiles per gather call
            NI = QT * P * K          # idxs per gather call (10240 / 5120)
            EW = O if last else 128  # gather elem_size in f16 units
            dparts = feat_state["dist"]
            fparts = feat_state["feat"]
            np_d = len(dparts)

            # ---- zRows: point-major gather source; staged to HBM so the
            # (non-transpose) dma_gather can fetch 2*EW-byte rows by point id.
            zr = zrpool.tile([P, 16, EW], F16, name=f"zr{li}", tag="zr")
            if O < EW:
                nc.gpsimd.memset(zr[:, :, O:EW], 0.0)
            for t in range(16):
                tc_cols = slice(t * P, (t + 1) * P)
                zq = zypsum.tile([P, O], F32, name=f"zq{li}_{t}", tag="zy")
                for pi, (lhsT, _) in enumerate(dparts):
                    nc.tensor.matmul(
                        zq[:], lhsT[:, tc_cols], wn_sb[li][pi][:],
                        start=(pi == 0), stop=(pi == np_d - 1),
                    )
                nc.scalar.copy(zr[:, t, 0:O], zq[:])
            zrh = dramp.tile([N, EW], F16, name=f"zrh{li}", tag="zrh")
            nc.sync.dma_start(
                zrh[:, :].rearrange("(t p) o -> p t o", p=P), zr[:]
            )

            # ---- yT = (Wc - Wn) @ feat for own queries: (nh, oc, NQ) f32
            yt = ypool.tile([P, nh, NQ], F32, name=f"yt{li}", tag="yt")
            for h in range(nh):
                o0 = h * P
                oc = min(O - o0, P)
                for ch in range(NQ // 512):
                    cs = bass.ts(ch, 512)
                    ypt = zypsum.tile([oc, 512], F32, name=f"yp{li}_{h}_{ch}",
                                      tag="zy")
                    for pi, fp in enumerate(fparts):
                        nc.tensor.matmul(
                            ypt[:], wd_sb[li][pi][:, o0:o0 + oc], fp[:, cs],
                            start=(pi == 0), stop=(pi == len(fparts) - 1),
                        )
                    nc.scalar.copy(yt[0:oc, h, cs], ypt[:])

            # ---- per-tile dist + packed top-k; gathers per tile in k-chunks
            # (<=1024 idxs per dma_gather: larger calls overflow the SWDGE
            # ring on this runtime). gather token i = 128*k_local + n%128:
            # gz[p, kl, :] = zrow[idx[n, k0+kl]]; the k-max is elementwise in
            # point-major layout, then PE transposes flip the reduced
            # (128, EW) tile result channel-major.
            KCH = [(0, 8), (8, 8), (16, 4)]  # (k0, KC) chunks per tile
            M = mpool.tile([P, nh, NQ], F16, name=f"M{li}", tag="m")
            M_rows = mpool.tile([P, NQT, EW], F16, name=f"Mr{li}", tag="mrows")
            ext = small.tile([P, K * NQT], I16, name=f"ext{li}", tag="ext")
            extv = ext[:, :].rearrange("p (t k) -> p t k", k=K)
            pending = []

            def emit_maxacc(tt, gzs):
                first = True
                for gz, (k0, KC) in zip(gzs, KCH):
                    for kl in range(KC):
                        s = gz[:, kl, :]
                        if first:
                            nc.vector.tensor_copy(out=M_rows[:, tt, :], in_=s)
                            first = False
                        else:
                            nc.vector.tensor_max(
                                M_rows[:, tt, :], M_rows[:, tt, :], s
                            )
                for h in range(nh):
                    tp = xxpsum.tile([P, P], F16, name=f"tp{li}_{tt}_{h}",
                                     tag="tp")
                    nc.tensor.transpose(
                        tp[:], M_rows[:, tt, h * P : (h + 1) * P], ident[:]
                    )
                    nc.scalar.copy(M[:, h, tt * P : (tt + 1) * P], tp[:])

            for t in range(NQT):
                tc_cols = slice(t * P, (t + 1) * P)
                qp = qpsum.tile([P, N], F32, name=f"q{li}_{t}", tag="q")
                for ch in range(4):
                    cs = bass.ts(ch, 512)
                    for pi, (lhsT, rhs) in enumerate(dparts):
                        nc.tensor.matmul(
                            qp[:, cs], lhsT[:, tc_cols], rhs[:, cs],
                            start=(pi == 0), stop=(pi == np_d - 1),
                        )
                # pack the column index into the low 11 mantissa bits
                qpk = qpool.tile([P, N], I32, name=f"qpk{li}_{t}", tag="qpk")
                nc.vector.scalar_tensor_tensor(
                    out=qpk[:], in0=qp[:].bitcast(I32), scalar=mask_hi[:],
                    in1=iota32[:], op0=AND, op1=OR,
                )
                qf = qpk[:].bitcast(F32)
                v8 = small.tile([P, 24], F32, name=f"v8{li}_{t}", tag="v8")
                for r in range(3):
                    nc.vector.max(out=v8[:, r * 8 : (r + 1) * 8], in_=qf)
                    if r < 2:
                        nc.vector.match_replace(
                            out=qf, in_to_replace=v8[:, r * 8 : (r + 1) * 8],
                            in_values=qf, imm_value=NEG,
                        )
                # top-20 indices = low 11 bits of the top-20 packed values
                # (bitwise TSP ops cannot cast, so extract i32 then cast i16)
                e32 = small.tile([P, K], I32, name=f"e32{li}_{t}", tag="e32")
                nc.vector.scalar_tensor_tensor(
                    out=e32[:], in0=v8[:, 0:K].bitcast(I32), scalar=mask_lo[:],
                    in1=zeros24[:, 0:K], op0=AND, op1=OR,
                )
                nc.vector.tensor_copy(
                    out=ext[:, t * K : (t + 1) * K], in_=e32[:]
                )

                if len(pending) > 4:
                    emit_maxacc(*pending.pop(0))
                if t % QT == QT - 1:
                    q = t // QT
                    T0, T1 = q * QT, (q + 1) * QT
                    W = QT * 160                 # idx cols per quarter
                    C0 = q * W
                    # fold 128 -> 16 partitions (contiguous 40B runs)
                    stg = small.tile([16, W], I16, name=f"stg{li}_{q}",
                                     tag="stg")
                    stgv = stg[0:16, :].rearrange(
                        "p (a t k) -> p a t k", a=8, k=K
                    )
                    for a in range(8):
                        nc.sync.dma_start(
                            stgv[:, a, :, :],
                            extv[16 * a : 16 * (a + 1), T0:T1, :],
                        )
                    # per (tile, k-chunk) call: interleave the idx values so
                    # token i's value sits at (row i%16, col 8*k_local + a),
                    # then gather. One replica DMA covers the quarter.
                    qgz = []
                    for tl in range(QT):
                        tt = T0 + tl
                        cb = 160 * tt
                        gzs = []
                        for k0, KC in KCH:
                            NI2 = 128 * KC
                            WC = 8 * KC
                            nc.vector.tensor_copy(
                                out=idxL[0:16, cb : cb + WC].rearrange(
                                    "p (s a) -> p s a", a=8
                                ),
                                in_=stg[0:16, :].rearrange(
                                    "p (a t k) -> p t k a", a=8, k=K
                                )[:, tl, k0 : k0 + KC, :],
                            )
                            cb += WC
                            gzs.append((k0, KC))
                        qgz.append((tt, gzs))
                    nc.sync.dma_start(
                        idxL[16:32, C0 : C0 + W], idxL[0:16, C0 : C0 + W]
                    )
                    for tt, gzs in qgz:
                        cb = 160 * tt
                        tiles = []
                        for k0, KC in gzs:
                            NI2 = 128 * KC
                            WC = 8 * KC
                            gz = gzp.tile([P, KC, EW], F16,
                                          name=f"gz{li}_{tt}_{k0}", tag="gz")
                            nc.gpsimd.dma_gather(
                                gz[:], zrh[:], idxL[:, cb : cb + WC],
                                NI2, NI2, EW,
                            )
                            cb += WC
                            tiles.append(gz)
                        pending.append((tt, tiles))
            while pending:
                emit_maxacc(*pending.pop(0))

            # ---- combine: out = lrelu(M + (y - z)) ----
            pre = prep.tile([P, nh, NQ], F32, name=f"pre{li}", tag="pre")
            ocm = min(O, P)
            nc.vector.tensor_add(pre[0:ocm], M[0:ocm], yt[0:ocm])
            # partial channel max over own points (first 1024 queries);
            # lrelu commutes with max, so reduce first, lrelu on (P,1)
            t1 = small.tile([P, nh], F32, name=f"t1_{li}", tag="t1")
            for h in range(nh):
                oc = min(O - h * P, P)
                col = {1: 0, 2: 1, 3: 2}.get(li, 3 + h)
                nc.vector.tensor_reduce(
                    out=t1[0:oc, h : h + 1], in_=pre[0:oc, h, 0:1024],
                    axis=AX, op=MAX,
                )
                nc.vector.scalar_tensor_tensor(
                    out=pm[0:oc, col : col + 1], in0=t1[0:oc, h : h + 1],
                    scalar=0.2, in1=t1[0:oc, h : h + 1], op0=MUL, op1=MAX,
                )
            if last:
                break

            # ---- featT for the next layer ----
            # aug rows: featT gets (-1, xx); augK gets (xx, -(1+2^-13))
            EPSN = -(1.0 + 2.0 ** -13)
            C2 = O
            if C2 <= 64:
                ft = feats.tile([C2 + 2, N], F32, name=f"ft{li+1}", tag="ft")
                nc.vector.scalar_tensor_tensor(
                    out=ft[0:C2, :], in0=pre[0:C2, 0, :], scalar=0.2,
                    in1=pre[0:C2, 0, :], op0=MUL, op1=MAX,
                )
                nc.vector.memset(ft[C2 : C2 + 2, :], -1.0)
                new_feat = [ft[0:C2, :]]
            else:
                sc = feats.tile([P, N], F32, name="sc", tag="sc", bufs=1)
                nc.vector.scalar_tensor_tensor(
                    out=sc[:], in0=pre[:, 0, :], scalar=0.2,
                    in1=pre[:, 0, :], op0=MUL, op1=MAX,
                )
                ftb = feats.tile([66, N], F32, name="ftb", tag="ftb", bufs=1)
                nc.sync.dma_start(ftb[0:64, :], sc[64:128, :])
                nc.vector.memset(ftb[64:66, :], -1.0)
                new_feat = [sc[0:64, :], ftb[0:64, :]]

            # ---- next-layer augK (2*feat rows + xx row + eps row) ----
            sq = prep.tile([P, N], F32, name=f"sq{li}", tag="pre")
            # (engine partition starts must be in {0,32,64,96}: memset both
            # aug rows from base 64, ACT-overwrite the xx row at 64, and DMA
            # the featT-side xx row, which sits at partition 65)
            if C2 <= 64:
                nc.scalar.square(sq[0:C2, :], ft[0:C2, :])
                ak = auks.tile([C2 + 2, N], F32, name=f"ak{li+1}", tag="ak")
                nc.scalar.mul(ak[0:C2, :], ft[0:C2, :], 2.0)
                nc.vector.memset(ak[C2 : C2 + 2, :], EPSN)
                for ch in range(4):
                    cs = bass.ts(ch, 512)
                    xp = xxpsum.tile([1, 512], F32, name=f"xx{li}_{ch}", tag="xx")
                    nc.tensor.matmul(xp[:], ones_col[0:C2, :], sq[0:C2, cs])
                    nc.scalar.copy(ak[C2 : C2 + 1, cs], xp[:])
                nc.sync.dma_start(ft[C2 + 1 : C2 + 2, :], ak[C2 : C2 + 1, :])
                feat_state = {"dist": [(ft[:], ak[:])], "feat": new_feat}
            else:
                nc.scalar.square(sq[0:C2, :], sc[:])
                aka = auks.tile([64, N], F32, name="aka", tag="aka")
                akb = auks.tile([66, N], F32, name="akb", tag="akb")
                nc.scalar.mul(aka[:], sc[0:64, :], 2.0)
                nc.scalar.mul(akb[0:64, :], ftb[0:64, :], 2.0)
                nc.vector.memset(akb[64:66, :], EPSN)
                for ch in range(4):
                    cs = bass.ts(ch, 512)
                    xp = xxpsum.tile([1, 512], F32, name=f"xx{li}_{ch}", tag="xx")
                    nc.tensor.matmul(xp[:], ones_col[0:C2, :], sq[0:C2, cs])
                    nc.scalar.copy(akb[64:65, cs], xp[:])
                nc.sync.dma_start(ftb[65:66, :], akb[64:65, :])
                feat_state = {
                    "dist": [(sc[0:64, :], aka[:]), (ftb[:], akb[:])],
                    "feat": new_feat,
                }

        nc.sync.dma_start(pmax_d[:], pm[:])


_NC_CACHE = None
TRACE = False          # set True (e.g. from test.py) to profile the HW run
RUN_KWARGS = {}        # extra kwargs for run_bass_kernel_spmd when tracing
LAST_RESULTS = None    # BassKernelResults of the most recent run


def _get_program():
    global _NC_CACHE
    if _NC_CACHE is None:
        nc = build_program()
        nc.finalize()   # bacc passes: library loads, act tables, ISA codegen
        _NC_CACHE = nc
    return _NC_CACHE


def make_in_maps(inputs):
    """Host-side prep: per-core featT1/augK1 + folded weight matrices."""
    x = np.asarray(inputs["x"], dtype=np.float32)       # (4, 2048, 3)
    ws = {i: np.asarray(inputs[f"w{i}"], np.float32) for i in (1, 2, 3, 4)}
    EPS = 1e-5
    wmats = {}
    for li, (C, O, _) in enumerate(LAYERS, start=1):
        g = np.asarray(inputs[f"g{li}"], np.float64)
        b = np.asarray(inputs[f"b{li}"], np.float64)
        m = np.asarray(inputs[f"m{li}"], np.float64)
        v = np.asarray(inputs[f"v{li}"], np.float64)
        s = (g / np.sqrt(v + EPS)).astype(np.float32)
        t = (b - m * s).astype(np.float32)
        assert np.all(s > 0) and np.allclose(t, 0.0), \
            "kernel assumes BN shift==0, scale>0"
        w = ws[li] * s[:, None]                           # fold BN scale
        wn = np.ascontiguousarray(w[:, :C].T)             # (C, O)
        wdm = np.ascontiguousarray((w[:, C:] - w[:, :C]).T)
        wmats[li] = (
            np.concatenate([wn, np.zeros((2, O), np.float32)], axis=0),
            wdm,
        )

    in_maps = []
    for core in range(8):
        bb = core // 2
        roll = (core % 2) * 1024
        xp = np.concatenate([x[bb, roll:], x[bb, :roll]], axis=0)  # (2048, 3)
        xx = np.sum(xp.astype(np.float32) ** 2, axis=1)
        featT1 = np.concatenate(
            [xp.T, np.full((1, N), -1.0, np.float32), xx[None, :]], axis=0
        ).astype(np.float32)
        augK1 = np.concatenate(
            [2.0 * xp.T, xx[None, :],
             np.full((1, N), -(1.0 + 2.0 ** -13), np.float32)], axis=0
        ).astype(np.float32)
        im = {"featT1": featT1, "augK1": augK1,
              "ident": np.eye(128, dtype=np.float16)}
        for li in range(1, len(LAYERS) + 1):
            im[f"wn{li}"] = wmats[li][0]
            im[f"wd{li}"] = wmats[li][1]
        in_maps.append(im)
    return in_maps


def assemble(results, B=4):
    # pm cols = [x1(64), x2(64), x3(128), x4a(128), x4b(128)]
    out = np.empty((B, 1, 512), np.float32)
    for b in range(B):
        vs = []
        for core in (2 * b, 2 * b + 1):
            pmv = results[core]["pmax"]
            vs.append(
                np.concatenate(
                    [pmv[0:64, 0], pmv[0:64, 1], pmv[0:128, 2], pmv[0:128, 3],
                     pmv[0:128, 4]]
                )
            )
        out[b, 0] = np.maximum(vs[0], vs[1])
    return out


def kernel(**inputs) -> np.ndarray:
    in_maps = make_in_maps(inputs)
    nc = _get_program()
    res = run_bass_kernel_spmd(
        nc, in_maps, core_ids=list(range(8)), trace=TRACE, **RUN_KWARGS
    )
    global LAST_RESULTS
    LAST_RESULTS = res
    return assemble(res.results, B=np.asarray(inputs["x"]).shape[0])


# revision 35
# speedup vs baseline: 3.7096x; 1.0944x over previous
"""DGCNN encoder (4x EdgeConv + global max) as a Bass/Tile kernel on 8 TRN2 cores.

Sharding: data-parallel over batch B=4 with a 2-way query split per cloud.
Core c handles cloud c//2. The host rotates each cloud's points by 1024 for
odd cores, so every core runs the SAME program: layers 1-3 are computed for
all 2048 points (needed for the next layer's kNN), layer 4 and the partial
channel max only for the first 1024 points (= this core's half). The final
(B,1,512) is a host-side max of the two per-core partial maxima per cloud.

Per-layer device algorithm (v2 — dma_gather + packed-index top-k):
  dist score q[n,j] = 2<f_n,f_j> - |f_j|^2  (selection-equivalent to the
  reference's top_k ordering; PE matmul with an augmented contraction row)
  -> pack point index into the low 11 mantissa bits of each fp32 score with
     ONE fused DVE op: qpk = (q & 0xFFFFF800) | iota  (scalar_tensor_tensor)
  -> exact top-20 via 3 rounds of DVE max8 (+ match_replace between rounds);
     indices come for free from the low bits of the max8 values (no
     find_index8 passes). 5 DVE passes/tile instead of 8.
  -> zRows (point-major (N, O) f16, padded rows of 512B stripes) is computed
     directly by PE matmuls (featT_tile^T @ WnT) — the gather source.
  -> neighbor gather+transpose runs on the 16 SDMA engines via
     gpsimd.dma_gather(transpose=True, SBUF source): ~1.8 ns/idx vs ~28 ns/idx
     for the old Q7 indirect_copy path. Output is channel-major (O, n_idx).
  -> M[o,n] = max_k z[o,idx[n,k]] via 20 strided DVE f16 maxes.
  -> y - z is ONE matmul with host-folded (Wc - Wn) weights; out =
     lrelu(M + (y-z)) via one DVE add + one fused mul/max (lrelu) pass.
"""
import numpy as np

import concourse.bass as bass
import concourse.tile as tile
from concourse import bacc, mybir
from concourse.bass_utils import run_bass_kernel_spmd

F32 = mybir.dt.float32
F16 = mybir.dt.float16
I32 = mybir.dt.int32
I16 = mybir.dt.int16

N = 2048          # points per cloud
K = 20            # neighbors
P = 128           # partitions
NEG = -1.0e30
AX = mybir.AxisListType.X
LAYERS = [
    # (Cin, Cout, n_qtiles)
    (3, 64, 16),
    (64, 64, 16),
    (64, 128, 16),
    (128, 256, 8),
]


def build_program():
    nc = bacc.Bacc("TRN2", target_bir_lowering=False, debug=False, num_devices=8,
                   num_swdge_queues=4)

    featT1 = nc.declare_dram_parameter("featT1", [5, N], F32, isOutput=False)
    augK1 = nc.declare_dram_parameter("augK1", [5, N], F32, isOutput=False)
    wn = {}
    wd = {}
    for li, (C, O, _) in enumerate(LAYERS, start=1):
        # wn: (C+2, O) with zero last rows (pair with featT's aug rows so the
        # zRows matmul can share the dist stationary parts)
        wn[li] = nc.declare_dram_parameter(f"wn{li}", [C + 2, O], F32, isOutput=False)
        # wd = (Wc - Wn)^T: y - z in one matmul
        wd[li] = nc.declare_dram_parameter(f"wd{li}", [C, O], F32, isOutput=False)
    ident = nc.declare_dram_parameter("ident", [P, P], F16, isOutput=False)
    pmax_out = nc.declare_dram_parameter("pmax", [P, 5], F32, isOutput=True)

    with tile.TileContext(nc) as tc:
        _build_tc(tc, featT1, augK1, wn, wd, ident, pmax_out)
    return nc


def _build_tc(tc, featT1_d, augK1_d, wn_d, wd_d, ident_d, pmax_d):
    nc = tc.nc
    import contextlib

    AND = mybir.AluOpType.bitwise_and
    OR = mybir.AluOpType.bitwise_or
    MUL = mybir.AluOpType.mult
    MAX = mybir.AluOpType.max

    with contextlib.ExitStack() as ctx:
        const = ctx.enter_context(tc.tile_pool(name="const", bufs=1))
        feats = ctx.enter_context(tc.tile_pool(name="feats", bufs=2))
        auks = ctx.enter_context(tc.tile_pool(name="auks", bufs=1))
        qpool = ctx.enter_context(tc.tile_pool(name="qpool", bufs=2))
        small = ctx.enter_context(tc.tile_pool(name="small", bufs=2))
        zrpool = ctx.enter_context(tc.tile_pool(name="zrpool", bufs=2))
        gzp = ctx.enter_context(tc.tile_pool(name="gzp", bufs=12))
        mpool = ctx.enter_context(tc.tile_pool(name="mpool", bufs=1))
        ypool = ctx.enter_context(tc.tile_pool(name="ypool", bufs=1))
        prep = ctx.enter_context(tc.tile_pool(name="prep", bufs=1))
        dramp = ctx.enter_context(tc.tile_pool(name="dramp", bufs=2, space="DRAM"))
        qpsum = ctx.enter_context(tc.tile_pool(name="qpsum", bufs=1, space="PSUM"))
        zypsum = ctx.enter_context(tc.tile_pool(name="zypsum", bufs=2, space="PSUM"))
        xxpsum = ctx.enter_context(tc.tile_pool(name="xxpsum", bufs=1, space="PSUM"))

        ones_col = const.tile([P, 1], F32, name="ones_col")
        nc.vector.memset(ones_col[:], 1.0)
        pm = const.tile([P, 5], F32, name="pm")
        nc.vector.memset(pm[:], NEG)
        mask_hi = const.tile([P, 1], I32, name="mask_hi")
        nc.vector.memset(mask_hi[:], -2048)        # 0xFFFFF800
        mask_lo = const.tile([P, 1], I32, name="mask_lo")
        nc.vector.memset(mask_lo[:], 2047)         # 0x000007FF
        zeros24 = const.tile([P, 24], I32, name="zeros24")
        nc.vector.memset(zeros24[:], 0)
        iota32 = const.tile([P, N], I32, name="iota32")
        nc.gpsimd.iota(iota32[:], pattern=[[1, N]], base=0, channel_multiplier=0)
        # dma_gather idx tensor: rows 0-15 data, 16-31 replica (Q7 pair for
        # queue 0); rows 32-127 must hold valid values for the simulator.
        idxL = const.tile([P, 2560], I16, name="idxL")
        nc.gpsimd.memset(idxL[:], 0)

        featT1 = const.tile([5, N], F32, name="featT1")
        nc.sync.dma_start(featT1[:], featT1_d[:])
        augK1 = const.tile([5, N], F32, name="augK1")
        nc.sync.dma_start(augK1[:], augK1_d[:])
        ident = const.tile([P, P], F16, name="ident")
        nc.sync.dma_start(ident[:], ident_d[:])

        # weights: contraction parts start at base partition 0 each
        wn_sb = {}
        wd_sb = {}
        for li, (C, O, _) in enumerate(LAYERS, start=1):
            if C <= 64:
                wsplit = [(0, C + 2)]
                dsplit = [(0, C)]
            else:
                wsplit = [(0, 64), (64, C + 2)]
                dsplit = [(0, 64), (64, C)]
            wn_sb[li] = []
            wd_sb[li] = []
            for ci, (c0, c1) in enumerate(wsplit):
                t = const.tile([c1 - c0, O], F32, name=f"wn{li}_{ci}")
                nc.sync.dma_start(t[:], wn_d[li][c0:c1, :])
                wn_sb[li].append(t)
            for ci, (c0, c1) in enumerate(dsplit):
                t = const.tile([c1 - c0, O], F32, name=f"wd{li}_{ci}")
                nc.sync.dma_start(t[:], wd_d[li][c0:c1, :])
                wd_sb[li].append(t)

        # dist: list of (lhsT_ap, rhs_ap) contraction parts. featT carries aug
        # rows (-1, xx[n]); augK carries (xx[j], -(1+eps)) so the matmul
        # yields q[n,j] = 2<f_n,f_j> - xx[j] - (1+eps)*xx[n] = -d^2 - eps*xx[n]
        # (row-constant shift: selection-equivalent, and top scores sit near 0
        # so clearing 11 mantissa bits for index packing costs ~2^-12 of the
        # per-candidate distance, not of the raw score magnitude).
        # feat: lhsT contraction parts without aug rows (for y-z)
        feat_state = {
            "dist": [(featT1[0:5, :], augK1[0:5, :])],
            "feat": [featT1[0:3, :]],
        }
        qctr = [0]  # round-robin SWDGE queue assignment for gathers

        for li, (C, O, NQT) in enumerate(LAYERS, start=1):
            last = li == 4
            NQ = NQT * P
            nh = 2 if last else 1
            QT = NQT // 4            # query tiles per gather call
            NI = QT * P * K          # idxs per gather call (10240 / 5120)
            EW = O if last else 128  # gather elem_size in f16 units
            dparts = feat_state["dist"]
            fparts = feat_state["feat"]
            np_d = len(dparts)

            # ---- zRows: point-major gather source; staged to HBM so the
            # (non-transpose) dma_gather can fetch 2*EW-byte rows by point id.
            zr = zrpool.tile([P, 16, EW], F16, name=f"zr{li}", tag="zr")
            if O < EW:
                nc.gpsimd.memset(zr[:, :, O:EW], 0.0)
            for t in range(16):
                tc_cols = slice(t * P, (t + 1) * P)
                zq = zypsum.tile([P, O], F32, name=f"zq{li}_{t}", tag="zy")
                for pi, (lhsT, _) in enumerate(dparts):
                    nc.tensor.matmul(
                        zq[:], lhsT[:, tc_cols], wn_sb[li][pi][:],
                        start=(pi == 0), stop=(pi == np_d - 1),
                    )
                nc.scalar.copy(zr[:, t, 0:O], zq[:])
            zrh = dramp.tile([N, EW], F16, name=f"zrh{li}", tag="zrh")
            nc.sync.dma_start(
                zrh[:, :].rearrange("(t p) o -> p t o", p=P), zr[:]
            )

            # ---- yT = (Wc - Wn) @ feat for own queries: (nh, oc, NQ) f32
            yt = ypool.tile([P, nh, NQ], F32, name=f"yt{li}", tag="yt")
            for h in range(nh):
                o0 = h * P
                oc = min(O - o0, P)
                for ch in range(NQ // 512):
                    cs = bass.ts(ch, 512)
                    ypt = zypsum.tile([oc, 512], F32, name=f"yp{li}_{h}_{ch}",
                                      tag="zy")
                    for pi, fp in enumerate(fparts):
                        nc.tensor.matmul(
                            ypt[:], wd_sb[li][pi][:, o0:o0 + oc], fp[:, cs],
                            start=(pi == 0), stop=(pi == len(fparts) - 1),
                        )
                    nc.scalar.copy(yt[0:oc, h, cs], ypt[:])

            # ---- per-tile dist + packed top-k; gathers per tile in k-chunks
            # (<=1024 idxs per dma_gather: larger calls overflow the SWDGE
            # ring on this runtime). gather token i = 128*k_local + n%128:
            # gz[p, kl, :] = zrow[idx[n, k0+kl]]; the k-max is elementwise in
            # point-major layout, then PE transposes flip the reduced
            # (128, EW) tile result channel-major.
            KCH = [(0, 8), (8, 8), (16, 4)]  # (k0, KC) chunks per tile
            M = mpool.tile([P, nh, NQ], F16, name=f"M{li}", tag="m")
            M_rows = mpool.tile([P, NQT, EW], F16, name=f"Mr{li}", tag="mrows")
            ext = small.tile([P, K * NQT], I16, name=f"ext{li}", tag="ext")
            extv = ext[:, :].rearrange("p (t k) -> p t k", k=K)
            pending = []

            def emit_maxacc(tt, gzs):
                first = True
                for gz, (k0, KC) in zip(gzs, KCH):
                    for kl in range(KC):
                        s = gz[:, kl, :]
                        if first:
                            nc.vector.tensor_copy(out=M_rows[:, tt, :], in_=s)
                            first = False
                        else:
                            nc.vector.tensor_max(
                                M_rows[:, tt, :], M_rows[:, tt, :], s
                            )
                for h in range(nh):
                    tp = xxpsum.tile([P, P], F16, name=f"tp{li}_{tt}_{h}",
                                     tag="tp")
                    nc.tensor.transpose(
                        tp[:], M_rows[:, tt, h * P : (h + 1) * P], ident[:]
                    )
                    nc.scalar.copy(M[:, h, tt * P : (tt + 1) * P], tp[:])

            for t in range(NQT):
                tc_cols = slice(t * P, (t + 1) * P)
                qp = qpsum.tile([P, N], F32, name=f"q{li}_{t}", tag="q")
                for ch in range(4):
                    cs = bass.ts(ch, 512)
                    for pi, (lhsT, rhs) in enumerate(dparts):
                        nc.tensor.matmul(
                            qp[:, cs], lhsT[:, tc_cols], rhs[:, cs],
                            start=(pi == 0), stop=(pi == np_d - 1),
                        )
                # pack the column index into the low 11 mantissa bits
                qpk = qpool.tile([P, N], I32, name=f"qpk{li}_{t}", tag="qpk")
                nc.vector.scalar_tensor_tensor(
                    out=qpk[:], in0=qp[:].bitcast(I32), scalar=mask_hi[:],
                    in1=iota32[:], op0=AND, op1=OR,
                )
                qf = qpk[:].bitcast(F32)
                v8 = small.tile([P, 24], F32, name=f"v8{li}_{t}", tag="v8")
                for r in range(3):
                    nc.vector.max(out=v8[:, r * 8 : (r + 1) * 8], in_=qf)
                    if r < 2:
                        nc.vector.match_replace(
                            out=qf, in_to_replace=v8[:, r * 8 : (r + 1) * 8],
                            in_values=qf, imm_value=NEG,
                        )
                # top-20 indices = low 11 bits of the top-20 packed values
                # (bitwise TSP ops cannot cast, so extract i32 then cast i16)
                e32 = small.tile([P, K], I32, name=f"e32{li}_{t}", tag="e32")
                nc.vector.scalar_tensor_tensor(
                    out=e32[:], in0=v8[:, 0:K].bitcast(I32), scalar=mask_lo[:],
                    in1=zeros24[:, 0:K], op0=AND, op1=OR,
                )
                nc.vector.tensor_copy(
                    out=ext[:, t * K : (t + 1) * K], in_=e32[:]
                )

                if len(pending) > 4:
                    emit_maxacc(*pending.pop(0))
                if t % QT == QT - 1:
                    q = t // QT
                    T0, T1 = q * QT, (q + 1) * QT
                    W = QT * 160                 # idx cols per quarter
                    C0 = q * W
                    # fold 128 -> 16 partitions (contiguous 40B runs)
                    stg = small.tile([16, W], I16, name=f"stg{li}_{q}",
                                     tag="stg")
                    stgv = stg[0:16, :].rearrange(
                        "p (a t k) -> p a t k", a=8, k=K
                    )
                    for a in range(8):
                        nc.sync.dma_start(
                            stgv[:, a, :, :],
                            extv[16 * a : 16 * (a + 1), T0:T1, :],
                        )
                    # ONE interleave per tile: chunk calls' idx cols are
                    # adjacent, so col = 8*k + a globally over k in [0,20).
                    for tl in range(QT):
                        tt = T0 + tl
                        cb = 160 * tt
                        nc.vector.tensor_copy(
                            out=idxL[0:16, cb : cb + 160].rearrange(
                                "p (s a) -> p s a", a=8
                            ),
                            in_=stg[0:16, :].rearrange(
                                "p (a t k) -> p t k a", a=8, k=K
                            )[:, tl, :, :],
                        )
                    # replicate idx rows to all 4 Q7 queue pairs (log doubling)
                    nc.sync.dma_start(
                        idxL[16:32, C0 : C0 + W], idxL[0:16, C0 : C0 + W]
                    )
                    nc.sync.dma_start(
                        idxL[32:64, C0 : C0 + W], idxL[0:32, C0 : C0 + W]
                    )
                    nc.sync.dma_start(
                        idxL[64:128, C0 : C0 + W], idxL[0:64, C0 : C0 + W]
                    )
                    for tl in range(QT):
                        tt = T0 + tl
                        cb = 160 * tt
                        tiles = []
                        for k0, KC in KCH:
                            NI2 = 128 * KC
                            WC = 8 * KC
                            gz = gzp.tile([P, KC, EW], F16,
                                          name=f"gz{li}_{tt}_{k0}", tag="gz")
                            nc.gpsimd.dma_gather(
                                gz[:], zrh[:], idxL[:, cb : cb + WC],
                                NI2, NI2, EW,
                                queue_num=qctr[0] % 4,
                            )
                            qctr[0] += 1
                            cb += WC
                            tiles.append(gz)
                        pending.append((tt, tiles))
            while pending:
                emit_maxacc(*pending.pop(0))

            # ---- combine: out = lrelu(M + (y - z)) ----
            pre = prep.tile([P, nh, NQ], F32, name=f"pre{li}", tag="pre")
            ocm = min(O, P)
            nc.vector.tensor_add(pre[0:ocm], M[0:ocm], yt[0:ocm])
            # partial channel max over own points (first 1024 queries);
            # lrelu commutes with max, so reduce first, lrelu on (P,1)
            t1 = small.tile([P, nh], F32, name=f"t1_{li}", tag="t1")
            for h in range(nh):
                oc = min(O - h * P, P)
                col = {1: 0, 2: 1, 3: 2}.get(li, 3 + h)
                nc.vector.tensor_reduce(
                    out=t1[0:oc, h : h + 1], in_=pre[0:oc, h, 0:1024],
                    axis=AX, op=MAX,
                )
                nc.vector.scalar_tensor_tensor(
                    out=pm[0:oc, col : col + 1], in0=t1[0:oc, h : h + 1],
                    scalar=0.2, in1=t1[0:oc, h : h + 1], op0=MUL, op1=MAX,
                )
            if last:
                break

            # ---- featT for the next layer ----
            # aug rows: featT gets (-1, xx); augK gets (xx, -(1+2^-13))
            EPSN = -(1.0 + 2.0 ** -13)
            C2 = O
            if C2 <= 64:
                ft = feats.tile([C2 + 2, N], F32, name=f"ft{li+1}", tag="ft")
                nc.vector.scalar_tensor_tensor(
                    out=ft[0:C2, :], in0=pre[0:C2, 0, :], scalar=0.2,
                    in1=pre[0:C2, 0, :], op0=MUL, op1=MAX,
                )
                nc.vector.memset(ft[C2 : C2 + 2, :], -1.0)
                new_feat = [ft[0:C2, :]]
            else:
                sc = feats.tile([P, N], F32, name="sc", tag="sc", bufs=1)
                nc.vector.scalar_tensor_tensor(
                    out=sc[:], in0=pre[:, 0, :], scalar=0.2,
                    in1=pre[:, 0, :], op0=MUL, op1=MAX,
                )
                ftb = feats.tile([66, N], F32, name="ftb", tag="ftb", bufs=1)
                nc.sync.dma_start(ftb[0:64, :], sc[64:128, :])
                nc.vector.memset(ftb[64:66, :], -1.0)
                new_feat = [sc[0:64, :], ftb[0:64, :]]

            # ---- next-layer augK (2*feat rows + xx row + eps row) ----
            sq = prep.tile([P, N], F32, name=f"sq{li}", tag="pre")
            # (engine partition starts must be in {0,32,64,96}: memset both
            # aug rows from base 64, ACT-overwrite the xx row at 64, and DMA
            # the featT-side xx row, which sits at partition 65)
            if C2 <= 64:
                nc.scalar.square(sq[0:C2, :], ft[0:C2, :])
                ak = auks.tile([C2 + 2, N], F32, name=f"ak{li+1}", tag="ak")
                nc.scalar.mul(ak[0:C2, :], ft[0:C2, :], 2.0)
                nc.vector.memset(ak[C2 : C2 + 2, :], EPSN)
                for ch in range(4):
                    cs = bass.ts(ch, 512)
                    xp = xxpsum.tile([1, 512], F32, name=f"xx{li}_{ch}", tag="xx")
                    nc.tensor.matmul(xp[:], ones_col[0:C2, :], sq[0:C2, cs])
                    nc.scalar.copy(ak[C2 : C2 + 1, cs], xp[:])
                nc.sync.dma_start(ft[C2 + 1 : C2 + 2, :], ak[C2 : C2 + 1, :])
                feat_state = {"dist": [(ft[:], ak[:])], "feat": new_feat}
            else:
                nc.scalar.square(sq[0:C2, :], sc[:])
                aka = auks.tile([64, N], F32, name="aka", tag="aka")
                akb = auks.tile([66, N], F32, name="akb", tag="akb")
                nc.scalar.mul(aka[:], sc[0:64, :], 2.0)
                nc.scalar.mul(akb[0:64, :], ftb[0:64, :], 2.0)
                nc.vector.memset(akb[64:66, :], EPSN)
                for ch in range(4):
                    cs = bass.ts(ch, 512)
                    xp = xxpsum.tile([1, 512], F32, name=f"xx{li}_{ch}", tag="xx")
                    nc.tensor.matmul(xp[:], ones_col[0:C2, :], sq[0:C2, cs])
                    nc.scalar.copy(akb[64:65, cs], xp[:])
                nc.sync.dma_start(ftb[65:66, :], akb[64:65, :])
                feat_state = {
                    "dist": [(sc[0:64, :], aka[:]), (ftb[:], akb[:])],
                    "feat": new_feat,
                }

        nc.sync.dma_start(pmax_d[:], pm[:])


_NC_CACHE = None
TRACE = False          # set True (e.g. from test.py) to profile the HW run
RUN_KWARGS = {}        # extra kwargs for run_bass_kernel_spmd when tracing
LAST_RESULTS = None    # BassKernelResults of the most recent run


def _get_program():
    global _NC_CACHE
    if _NC_CACHE is None:
        nc = build_program()
        nc.finalize()   # bacc passes: library loads, act tables, ISA codegen
        _NC_CACHE = nc
    return _NC_CACHE


def make_in_maps(inputs):
    """Host-side prep: per-core featT1/augK1 + folded weight matrices."""
    x = np.asarray(inputs["x"], dtype=np.float32)       # (4, 2048, 3)
    ws = {i: np.asarray(inputs[f"w{i}"], np.float32) for i in (1, 2, 3, 4)}
    EPS = 1e-5
    wmats = {}
    for li, (C, O, _) in enumerate(LAYERS, start=1):
        g = np.asarray(inputs[f"g{li}"], np.float64)
        b = np.asarray(inputs[f"b{li}"], np.float64)
        m = np.asarray(inputs[f"m{li}"], np.float64)
        v = np.asarray(inputs[f"v{li}"], np.float64)
        s = (g / np.sqrt(v + EPS)).astype(np.float32)
        t = (b - m * s).astype(np.float32)
        assert np.all(s > 0) and np.allclose(t, 0.0), \
            "kernel assumes BN shift==0, scale>0"
        w = ws[li] * s[:, None]                           # fold BN scale
        wn = np.ascontiguousarray(w[:, :C].T)             # (C, O)
        wdm = np.ascontiguousarray((w[:, C:] - w[:, :C]).T)
        wmats[li] = (
            np.concatenate([wn, np.zeros((2, O), np.float32)], axis=0),
            wdm,
        )

    in_maps = []
    for core in range(8):
        bb = core // 2
        roll = (core % 2) * 1024
        xp = np.concatenate([x[bb, roll:], x[bb, :roll]], axis=0)  # (2048, 3)
        xx = np.sum(xp.astype(np.float32) ** 2, axis=1)
        featT1 = np.concatenate(
            [xp.T, np.full((1, N), -1.0, np.float32), xx[None, :]], axis=0
        ).astype(np.float32)
        augK1 = np.concatenate(
            [2.0 * xp.T, xx[None, :],
             np.full((1, N), -(1.0 + 2.0 ** -13), np.float32)], axis=0
        ).astype(np.float32)
        im = {"featT1": featT1, "augK1": augK1,
              "ident": np.eye(128, dtype=np.float16)}
        for li in range(1, len(LAYERS) + 1):
            im[f"wn{li}"] = wmats[li][0]
            im[f"wd{li}"] = wmats[li][1]
        in_maps.append(im)
    return in_maps


def assemble(results, B=4):
    # pm cols = [x1(64), x2(64), x3(128), x4a(128), x4b(128)]
    out = np.empty((B, 1, 512), np.float32)
    for b in range(B):
        vs = []
        for core in (2 * b, 2 * b + 1):
            pmv = results[core]["pmax"]
            vs.append(
                np.concatenate(
                    [pmv[0:64, 0], pmv[0:64, 1], pmv[0:128, 2], pmv[0:128, 3],
                     pmv[0:128, 4]]
                )
            )
        out[b, 0] = np.maximum(vs[0], vs[1])
    return out


def kernel(**inputs) -> np.ndarray:
    in_maps = make_in_maps(inputs)
    nc = _get_program()
    res = run_bass_kernel_spmd(
        nc, in_maps, core_ids=list(range(8)), trace=TRACE, **RUN_KWARGS
    )
    global LAST_RESULTS
    LAST_RESULTS = res
    return assemble(res.results, B=np.asarray(inputs["x"]).shape[0])


# revision 36
# speedup vs baseline: 4.5019x; 1.2136x over previous
"""DGCNN encoder (4x EdgeConv + global max) as a Bass/Tile kernel on 8 TRN2 cores.

Sharding: data-parallel over batch B=4 with a 2-way query split per cloud.
Core c handles cloud c//2. The host rotates each cloud's points by 1024 for
odd cores, so every core runs the SAME program: layers 1-3 are computed for
all 2048 points (needed for the next layer's kNN), layer 4 and the partial
channel max only for the first 1024 points (= this core's half). The final
(B,1,512) is a host-side max of the two per-core partial maxima per cloud.

Per-layer device algorithm (v2 — dma_gather + packed-index top-k):
  dist score q[n,j] = 2<f_n,f_j> - |f_j|^2  (selection-equivalent to the
  reference's top_k ordering; PE matmul with an augmented contraction row)
  -> pack point index into the low 11 mantissa bits of each fp32 score with
     ONE fused DVE op: qpk = (q & 0xFFFFF800) | iota  (scalar_tensor_tensor)
  -> exact top-20 via 3 rounds of DVE max8 (+ match_replace between rounds);
     indices come for free from the low bits of the max8 values (no
     find_index8 passes). 5 DVE passes/tile instead of 8.
  -> zRows (point-major (N, O) f16, padded rows of 512B stripes) is computed
     directly by PE matmuls (featT_tile^T @ WnT) — the gather source.
  -> neighbor gather+transpose runs on the 16 SDMA engines via
     gpsimd.dma_gather(transpose=True, SBUF source): ~1.8 ns/idx vs ~28 ns/idx
     for the old Q7 indirect_copy path. Output is channel-major (O, n_idx).
  -> M[o,n] = max_k z[o,idx[n,k]] via 20 strided DVE f16 maxes.
  -> y - z is ONE matmul with host-folded (Wc - Wn) weights; out =
     lrelu(M + (y-z)) via one DVE add + one fused mul/max (lrelu) pass.
"""
import numpy as np

import concourse.bass as bass
import concourse.tile as tile
from concourse import bacc, mybir
from concourse.bass_utils import run_bass_kernel_spmd

F32 = mybir.dt.float32
F16 = mybir.dt.float16
I32 = mybir.dt.int32
I16 = mybir.dt.int16

N = 2048          # points per cloud
K = 20            # neighbors
P = 128           # partitions
NEG = -1.0e30
AX = mybir.AxisListType.X
LAYERS = [
    # (Cin, Cout, n_qtiles)
    (3, 64, 16),
    (64, 64, 16),
    (64, 128, 16),
    (128, 256, 8),
]


def build_program():
    nc = bacc.Bacc("TRN2", target_bir_lowering=False, debug=False, num_devices=8,
                   num_swdge_queues=4)

    featT1 = nc.declare_dram_parameter("featT1", [5, N], F32, isOutput=False)
    augK1 = nc.declare_dram_parameter("augK1", [5, N], F32, isOutput=False)
    wn = {}
    wd = {}
    for li, (C, O, _) in enumerate(LAYERS, start=1):
        # wn: (C+2, O) with zero last rows (pair with featT's aug rows so the
        # zRows matmul can share the dist stationary parts)
        wn[li] = nc.declare_dram_parameter(f"wn{li}", [C + 2, O], F32, isOutput=False)
        # wd = (Wc - Wn)^T: y - z in one matmul
        wd[li] = nc.declare_dram_parameter(f"wd{li}", [C, O], F32, isOutput=False)
    ident = nc.declare_dram_parameter("ident", [P, P], F16, isOutput=False)
    pmax_out = nc.declare_dram_parameter("pmax", [P, 5], F32, isOutput=True)

    with tile.TileContext(nc) as tc:
        _build_tc(tc, featT1, augK1, wn, wd, ident, pmax_out)
    return nc


def _build_tc(tc, featT1_d, augK1_d, wn_d, wd_d, ident_d, pmax_d):
    nc = tc.nc
    import contextlib

    AND = mybir.AluOpType.bitwise_and
    OR = mybir.AluOpType.bitwise_or
    MUL = mybir.AluOpType.mult
    MAX = mybir.AluOpType.max

    with contextlib.ExitStack() as ctx:
        const = ctx.enter_context(tc.tile_pool(name="const", bufs=1))
        feats = ctx.enter_context(tc.tile_pool(name="feats", bufs=2))
        auks = ctx.enter_context(tc.tile_pool(name="auks", bufs=1))
        qpool = ctx.enter_context(tc.tile_pool(name="qpool", bufs=2))
        small = ctx.enter_context(tc.tile_pool(name="small", bufs=2))
        zrpool = ctx.enter_context(tc.tile_pool(name="zrpool", bufs=2))
        gzp = ctx.enter_context(tc.tile_pool(name="gzp", bufs=12))
        mpool = ctx.enter_context(tc.tile_pool(name="mpool", bufs=1))
        ypool = ctx.enter_context(tc.tile_pool(name="ypool", bufs=1))
        prep = ctx.enter_context(tc.tile_pool(name="prep", bufs=1))
        dramp = ctx.enter_context(tc.tile_pool(name="dramp", bufs=2, space="DRAM"))
        qpsum = ctx.enter_context(tc.tile_pool(name="qpsum", bufs=1, space="PSUM"))
        zypsum = ctx.enter_context(tc.tile_pool(name="zypsum", bufs=2, space="PSUM"))
        xxpsum = ctx.enter_context(tc.tile_pool(name="xxpsum", bufs=1, space="PSUM"))

        ones_col = const.tile([P, 1], F32, name="ones_col")
        nc.vector.memset(ones_col[:], 1.0)
        pm = const.tile([P, 5], F32, name="pm")
        nc.vector.memset(pm[:], NEG)
        mask_hi = const.tile([P, 1], I32, name="mask_hi")
        nc.vector.memset(mask_hi[:], -2048)        # 0xFFFFF800
        mask_lo = const.tile([P, 1], I32, name="mask_lo")
        nc.vector.memset(mask_lo[:], 2047)         # 0x000007FF
        zeros24 = const.tile([P, 24], I32, name="zeros24")
        nc.vector.memset(zeros24[:], 0)
        iota32 = const.tile([P, N], I32, name="iota32")
        nc.gpsimd.iota(iota32[:], pattern=[[1, N]], base=0, channel_multiplier=0)
        # dma_gather idx tensor: rows 0-15 data, 16-31 replica (Q7 pair for
        # queue 0); rows 32-127 must hold valid values for the simulator.
        idxL = const.tile([P, 2560], I16, name="idxL")
        nc.gpsimd.memset(idxL[:], 0)

        featT1 = const.tile([5, N], F32, name="featT1")
        nc.sync.dma_start(featT1[:], featT1_d[:])
        augK1 = const.tile([5, N], F32, name="augK1")
        nc.sync.dma_start(augK1[:], augK1_d[:])
        ident = const.tile([P, P], F16, name="ident")
        nc.sync.dma_start(ident[:], ident_d[:])

        # weights: contraction parts start at base partition 0 each
        wn_sb = {}
        wd_sb = {}
        for li, (C, O, _) in enumerate(LAYERS, start=1):
            if C <= 64:
                wsplit = [(0, C + 2)]
                dsplit = [(0, C)]
            else:
                wsplit = [(0, 64), (64, C + 2)]
                dsplit = [(0, 64), (64, C)]
            wn_sb[li] = []
            wd_sb[li] = []
            for ci, (c0, c1) in enumerate(wsplit):
                t = const.tile([c1 - c0, O], F32, name=f"wn{li}_{ci}")
                nc.sync.dma_start(t[:], wn_d[li][c0:c1, :])
                wn_sb[li].append(t)
            for ci, (c0, c1) in enumerate(dsplit):
                t = const.tile([c1 - c0, O], F32, name=f"wd{li}_{ci}")
                nc.sync.dma_start(t[:], wd_d[li][c0:c1, :])
                wd_sb[li].append(t)

        # dist: list of (lhsT_ap, rhs_ap) contraction parts. featT carries aug
        # rows (-1, xx[n]); augK carries (xx[j], -(1+eps)) so the matmul
        # yields q[n,j] = 2<f_n,f_j> - xx[j] - (1+eps)*xx[n] = -d^2 - eps*xx[n]
        # (row-constant shift: selection-equivalent, and top scores sit near 0
        # so clearing 11 mantissa bits for index packing costs ~2^-12 of the
        # per-candidate distance, not of the raw score magnitude).
        # feat: lhsT contraction parts without aug rows (for y-z)
        feat_state = {
            "dist": [(featT1[0:5, :], augK1[0:5, :])],
            "feat": [featT1[0:3, :]],
        }
        qctr = [0]  # round-robin SWDGE queue assignment for gathers

        for li, (C, O, NQT) in enumerate(LAYERS, start=1):
            last = li == 4
            NQ = NQT * P
            nh = 2 if last else 1
            QT = NQT // 4            # query tiles per gather call
            NI = QT * P * K          # idxs per gather call (10240 / 5120)
            EW = O if last else 128  # gather elem_size in f16 units
            dparts = feat_state["dist"]
            fparts = feat_state["feat"]
            np_d = len(dparts)

            # ---- zRows: point-major gather source; staged to HBM so the
            # (non-transpose) dma_gather can fetch 2*EW-byte rows by point id.
            zr = zrpool.tile([P, 16, EW], F16, name=f"zr{li}", tag="zr")
            if O < EW:
                nc.gpsimd.memset(zr[:, :, O:EW], 0.0)
            for t in range(16):
                tc_cols = slice(t * P, (t + 1) * P)
                zq = zypsum.tile([P, O], F32, name=f"zq{li}_{t}", tag="zy")
                for pi, (lhsT, _) in enumerate(dparts):
                    nc.tensor.matmul(
                        zq[:], lhsT[:, tc_cols], wn_sb[li][pi][:],
                        start=(pi == 0), stop=(pi == np_d - 1),
                    )
                nc.scalar.copy(zr[:, t, 0:O], zq[:])
            zrh = dramp.tile([N, EW], F16, name=f"zrh{li}", tag="zrh")
            nc.sync.dma_start(
                zrh[:, :].rearrange("(t p) o -> p t o", p=P), zr[:]
            )

            # ---- yT = (Wc - Wn) @ feat for own queries: (nh, oc, NQ) f32
            yt = ypool.tile([P, nh, NQ], F32, name=f"yt{li}", tag="yt")
            for h in range(nh):
                o0 = h * P
                oc = min(O - o0, P)
                for ch in range(NQ // 512):
                    cs = bass.ts(ch, 512)
                    ypt = zypsum.tile([oc, 512], F32, name=f"yp{li}_{h}_{ch}",
                                      tag="zy")
                    for pi, fp in enumerate(fparts):
                        nc.tensor.matmul(
                            ypt[:], wd_sb[li][pi][:, o0:o0 + oc], fp[:, cs],
                            start=(pi == 0), stop=(pi == len(fparts) - 1),
                        )
                    nc.scalar.copy(yt[0:oc, h, cs], ypt[:])

            # ---- per-tile dist + packed top-k; gathers per tile in k-chunks
            # (<=1024 idxs per dma_gather: larger calls overflow the SWDGE
            # ring on this runtime). gather token i = 128*k_local + n%128:
            # gz[p, kl, :] = zrow[idx[n, k0+kl]]; the k-max is elementwise in
            # point-major layout, then PE transposes flip the reduced
            # (128, EW) tile result channel-major.
            KCH = [(0, 8), (8, 8), (16, 4)]  # (k0, KC) chunks per tile
            M = mpool.tile([P, nh, NQ], F16, name=f"M{li}", tag="m")
            M_rows = mpool.tile([P, NQT, EW], F16, name=f"Mr{li}", tag="mrows")
            ext = small.tile([P, K * NQT], I32, name=f"ext{li}", tag="ext")
            extv = ext[:, :].rearrange("p (t k) -> p t k", k=K)
            pending = []

            def emit_maxacc(tt, gzs):
                first = True
                for gz, (k0, KC) in zip(gzs, KCH):
                    for kl in range(KC):
                        s = gz[:, kl, :]
                        if first:
                            nc.vector.tensor_copy(out=M_rows[:, tt, :], in_=s)
                            first = False
                        else:
                            nc.vector.tensor_max(
                                M_rows[:, tt, :], M_rows[:, tt, :], s
                            )
                for h in range(nh):
                    tp = xxpsum.tile([P, P], F16, name=f"tp{li}_{tt}_{h}",
                                     tag="tp")
                    nc.tensor.transpose(
                        tp[:], M_rows[:, tt, h * P : (h + 1) * P], ident[:]
                    )
                    nc.scalar.copy(M[:, h, tt * P : (tt + 1) * P], tp[:])

            for t in range(NQT):
                tc_cols = slice(t * P, (t + 1) * P)
                qp = qpsum.tile([P, N], F32, name=f"q{li}_{t}", tag="q")
                for ch in range(4):
                    cs = bass.ts(ch, 512)
                    for pi, (lhsT, rhs) in enumerate(dparts):
                        nc.tensor.matmul(
                            qp[:, cs], lhsT[:, tc_cols], rhs[:, cs],
                            start=(pi == 0), stop=(pi == np_d - 1),
                        )
                # pack the column index into the low 11 mantissa bits
                qpk = qpool.tile([P, N], I32, name=f"qpk{li}_{t}", tag="qpk")
                nc.vector.scalar_tensor_tensor(
                    out=qpk[:], in0=qp[:].bitcast(I32), scalar=mask_hi[:],
                    in1=iota32[:], op0=AND, op1=OR,
                )
                qf = qpk[:].bitcast(F32)
                v8 = small.tile([P, 24], F32, name=f"v8{li}_{t}", tag="v8")
                for r in range(3):
                    nc.vector.max(out=v8[:, r * 8 : (r + 1) * 8], in_=qf)
                    if r < 2:
                        nc.vector.match_replace(
                            out=qf, in_to_replace=v8[:, r * 8 : (r + 1) * 8],
                            in_values=qf, imm_value=NEG,
                        )
                # top-20 indices = low 11 bits of the top-20 packed values
                # (bitwise TSP ops cannot cast, so extract i32 then cast i16)
                nc.vector.scalar_tensor_tensor(
                    out=ext[:, t * K : (t + 1) * K],
                    in0=v8[:, 0:K].bitcast(I32), scalar=mask_lo[:],
                    in1=zeros24[:, 0:K], op0=AND, op1=OR,
                )

                if len(pending) > 4:
                    emit_maxacc(*pending.pop(0))
                if t % QT == QT - 1:
                    q = t // QT
                    T0, T1 = q * QT, (q + 1) * QT
                    W = QT * 160                 # idx cols per quarter
                    C0 = q * W
                    # fold 128 -> 16 partitions (contiguous 40B runs)
                    stg = small.tile([16, W], I32, name=f"stg{li}_{q}",
                                     tag="stg")
                    stgv = stg[0:16, :].rearrange(
                        "p (a t k) -> p a t k", a=8, k=K
                    )
                    for a in range(8):
                        nc.sync.dma_start(
                            stgv[:, a, :, :],
                            extv[16 * a : 16 * (a + 1), T0:T1, :],
                        )
                    # ONE interleave per tile: chunk calls' idx cols are
                    # adjacent, so col = 8*k + a globally over k in [0,20).
                    for tl in range(QT):
                        tt = T0 + tl
                        cb = 160 * tt
                        nc.vector.tensor_copy(
                            out=idxL[0:16, cb : cb + 160].rearrange(
                                "p (s a) -> p s a", a=8
                            ),
                            in_=stg[0:16, :].rearrange(
                                "p (a t k) -> p t k a", a=8, k=K
                            )[:, tl, :, :],
                        )
                    # replicate idx rows to all 4 Q7 queue pairs (log doubling)
                    nc.sync.dma_start(
                        idxL[16:32, C0 : C0 + W], idxL[0:16, C0 : C0 + W]
                    )
                    nc.sync.dma_start(
                        idxL[32:64, C0 : C0 + W], idxL[0:32, C0 : C0 + W]
                    )
                    nc.sync.dma_start(
                        idxL[64:128, C0 : C0 + W], idxL[0:64, C0 : C0 + W]
                    )
                    for tl in range(QT):
                        tt = T0 + tl
                        cb = 160 * tt
                        tiles = []
                        for k0, KC in KCH:
                            NI2 = 128 * KC
                            WC = 8 * KC
                            gz = gzp.tile([P, KC, EW], F16,
                                          name=f"gz{li}_{tt}_{k0}", tag="gz")
                            nc.gpsimd.dma_gather(
                                gz[:], zrh[:], idxL[:, cb : cb + WC],
                                NI2, NI2, EW,
                                queue_num=qctr[0] % 4,
                            )
                            qctr[0] += 1
                            cb += WC
                            tiles.append(gz)
                        pending.append((tt, tiles))
            while pending:
                emit_maxacc(*pending.pop(0))

            # ---- combine: out = lrelu(M + (y - z)) ----
            pre = prep.tile([P, nh, NQ], F32, name=f"pre{li}", tag="pre")
            ocm = min(O, P)
            nc.vector.tensor_add(pre[0:ocm], M[0:ocm], yt[0:ocm])
            # partial channel max over own points (first 1024 queries);
            # lrelu commutes with max, so reduce first, lrelu on (P,1)
            t1 = small.tile([P, nh], F32, name=f"t1_{li}", tag="t1")
            for h in range(nh):
                oc = min(O - h * P, P)
                col = {1: 0, 2: 1, 3: 2}.get(li, 3 + h)
                nc.vector.tensor_reduce(
                    out=t1[0:oc, h : h + 1], in_=pre[0:oc, h, 0:1024],
                    axis=AX, op=MAX,
                )
                nc.vector.scalar_tensor_tensor(
                    out=pm[0:oc, col : col + 1], in0=t1[0:oc, h : h + 1],
                    scalar=0.2, in1=t1[0:oc, h : h + 1], op0=MUL, op1=MAX,
                )
            if last:
                break

            # ---- featT for the next layer ----
            # aug rows: featT gets (-1, xx); augK gets (xx, -(1+2^-13))
            EPSN = -(1.0 + 2.0 ** -13)
            C2 = O
            if C2 <= 64:
                ft = feats.tile([C2 + 2, N], F32, name=f"ft{li+1}", tag="ft")
                nc.vector.scalar_tensor_tensor(
                    out=ft[0:C2, :], in0=pre[0:C2, 0, :], scalar=0.2,
                    in1=pre[0:C2, 0, :], op0=MUL, op1=MAX,
                )
                nc.vector.memset(ft[C2 : C2 + 2, :], -1.0)
                new_feat = [ft[0:C2, :]]
            else:
                sc = feats.tile([P, N], F32, name="sc", tag="sc", bufs=1)
                nc.vector.scalar_tensor_tensor(
                    out=sc[:], in0=pre[:, 0, :], scalar=0.2,
                    in1=pre[:, 0, :], op0=MUL, op1=MAX,
                )
                ftb = feats.tile([66, N], F32, name="ftb", tag="ftb", bufs=1)
                nc.sync.dma_start(ftb[0:64, :], sc[64:128, :])
                nc.vector.memset(ftb[64:66, :], -1.0)
                new_feat = [sc[0:64, :], ftb[0:64, :]]

            # ---- next-layer augK (2*feat rows + xx row + eps row) ----
            sq = prep.tile([P, N], F32, name=f"sq{li}", tag="pre")
            # (engine partition starts must be in {0,32,64,96}: memset both
            # aug rows from base 64, ACT-overwrite the xx row at 64, and DMA
            # the featT-side xx row, which sits at partition 65)
            if C2 <= 64:
                nc.scalar.square(sq[0:C2, :], ft[0:C2, :])
                ak = auks.tile([C2 + 2, N], F32, name=f"ak{li+1}", tag="ak")
                nc.scalar.mul(ak[0:C2, :], ft[0:C2, :], 2.0)
                nc.vector.memset(ak[C2 : C2 + 2, :], EPSN)
                for ch in range(4):
                    cs = bass.ts(ch, 512)
                    xp = xxpsum.tile([1, 512], F32, name=f"xx{li}_{ch}", tag="xx")
                    nc.tensor.matmul(xp[:], ones_col[0:C2, :], sq[0:C2, cs])
                    nc.scalar.copy(ak[C2 : C2 + 1, cs], xp[:])
                nc.sync.dma_start(ft[C2 + 1 : C2 + 2, :], ak[C2 : C2 + 1, :])
                feat_state = {"dist": [(ft[:], ak[:])], "feat": new_feat}
            else:
                nc.scalar.square(sq[0:C2, :], sc[:])
                aka = auks.tile([64, N], F32, name="aka", tag="aka")
                akb = auks.tile([66, N], F32, name="akb", tag="akb")
                nc.scalar.mul(aka[:], sc[0:64, :], 2.0)
                nc.scalar.mul(akb[0:64, :], ftb[0:64, :], 2.0)
                nc.vector.memset(akb[64:66, :], EPSN)
                for ch in range(4):
                    cs = bass.ts(ch, 512)
                    xp = xxpsum.tile([1, 512], F32, name=f"xx{li}_{ch}", tag="xx")
                    nc.tensor.matmul(xp[:], ones_col[0:C2, :], sq[0:C2, cs])
                    nc.scalar.copy(akb[64:65, cs], xp[:])
                nc.sync.dma_start(ftb[65:66, :], akb[64:65, :])
                feat_state = {
                    "dist": [(sc[0:64, :], aka[:]), (ftb[:], akb[:])],
                    "feat": new_feat,
                }

        nc.sync.dma_start(pmax_d[:], pm[:])


_NC_CACHE = None
TRACE = False          # set True (e.g. from test.py) to profile the HW run
RUN_KWARGS = {}        # extra kwargs for run_bass_kernel_spmd when tracing
LAST_RESULTS = None    # BassKernelResults of the most recent run


def _get_program():
    global _NC_CACHE
    if _NC_CACHE is None:
        nc = build_program()
        nc.finalize()   # bacc passes: library loads, act tables, ISA codegen
        _NC_CACHE = nc
    return _NC_CACHE


def make_in_maps(inputs):
    """Host-side prep: per-core featT1/augK1 + folded weight matrices."""
    x = np.asarray(inputs["x"], dtype=np.float32)       # (4, 2048, 3)
    ws = {i: np.asarray(inputs[f"w{i}"], np.float32) for i in (1, 2, 3, 4)}
    EPS = 1e-5
    wmats = {}
    for li, (C, O, _) in enumerate(LAYERS, start=1):
        g = np.asarray(inputs[f"g{li}"], np.float64)
        b = np.asarray(inputs[f"b{li}"], np.float64)
        m = np.asarray(inputs[f"m{li}"], np.float64)
        v = np.asarray(inputs[f"v{li}"], np.float64)
        s = (g / np.sqrt(v + EPS)).astype(np.float32)
        t = (b - m * s).astype(np.float32)
        assert np.all(s > 0) and np.allclose(t, 0.0), \
            "kernel assumes BN shift==0, scale>0"
        w = ws[li] * s[:, None]                           # fold BN scale
        wn = np.ascontiguousarray(w[:, :C].T)             # (C, O)
        wdm = np.ascontiguousarray((w[:, C:] - w[:, :C]).T)
        wmats[li] = (
            np.concatenate([wn, np.zeros((2, O), np.float32)], axis=0),
            wdm,
        )

    in_maps = []
    for core in range(8):
        bb = core // 2
        roll = (core % 2) * 1024
        xp = np.concatenate([x[bb, roll:], x[bb, :roll]], axis=0)  # (2048, 3)
        xx = np.sum(xp.astype(np.float32) ** 2, axis=1)
        featT1 = np.concatenate(
            [xp.T, np.full((1, N), -1.0, np.float32), xx[None, :]], axis=0
        ).astype(np.float32)
        augK1 = np.concatenate(
            [2.0 * xp.T, xx[None, :],
             np.full((1, N), -(1.0 + 2.0 ** -13), np.float32)], axis=0
        ).astype(np.float32)
        im = {"featT1": featT1, "augK1": augK1,
              "ident": np.eye(128, dtype=np.float16)}
        for li in range(1, len(LAYERS) + 1):
            im[f"wn{li}"] = wmats[li][0]
            im[f"wd{li}"] = wmats[li][1]
        in_maps.append(im)
    return in_maps


def assemble(results, B=4):
    # pm cols = [x1(64), x2(64), x3(128), x4a(128), x4b(128)]
    out = np.empty((B, 1, 512), np.float32)
    for b in range(B):
        vs = []
        for core in (2 * b, 2 * b + 1):
            pmv = results[core]["pmax"]
            vs.append(
                np.concatenate(
                    [pmv[0:64, 0], pmv[0:64, 1], pmv[0:128, 2], pmv[0:128, 3],
                     pmv[0:128, 4]]
                )
            )
        out[b, 0] = np.maximum(vs[0], vs[1])
    return out


def kernel(**inputs) -> np.ndarray:
    in_maps = make_in_maps(inputs)
    nc = _get_program()
    res = run_bass_kernel_spmd(
        nc, in_maps, core_ids=list(range(8)), trace=TRACE, **RUN_KWARGS
    )
    global LAST_RESULTS
    LAST_RESULTS = res
    return assemble(res.results, B=np.asarray(inputs["x"]).shape[0])
